# revision 14
# baseline (speedup 1.0000x reference)
"""GCNConv forward on 8 Trainium2 NeuronCores (Bass/Tile).

Strategy (graph/edge-cut parallelism):
  - Nodes are split into 784 buckets of 128 (98 buckets per core); each core
    owns the scatter-sum for its node shard.
  - norm = (1+outdeg)^-0.5 is pure graph-structure metadata, precomputed on
    host alongside the edge partitioning (same category as the index streams).
  - g = norm[src] * x is computed distributed (own rows only) and shared with
    an AllGather (bf16, Shared scratchpad), so per-edge messages are plain
    row-gathers of g.
  - Edges are grouped by (dst-bucket-group, src-chunk) on host; one dma_gather
    per (group, chunk) fetches all needed g rows (int16 indices limit a chunk
    to 25088 rows), amortizing SWDGE setup over ~1-2k descriptors.
  - Each 128-edge tile scatter-adds into its bucket's PSUM via a one-hot
    matmul; all one-hot tiles of a group are built in ONE DVE op using
    zero-stride broadcast APs (iota vs dst-slot).
  - Self-loops use the dense path: g rows for own nodes stay resident in SBUF
    from phase B and enter each bucket's PSUM chain via an identity matmul.
  - 4 buckets share one PSUM bank, so one wide Activation copy moves 4
    buckets' aggregates (and later 4 matmul results) at once.
  - norm[dst] scaling cancels inside the final L2 normalization (deg >= 1
    because of self-loops), so it is skipped entirely.
  - out = tanh(L2-normalize(agg @ W)) with the L2/rsqrt done via Ln/Exp on
    the scalar engine (Rsqrt activation is banned for accuracy).
"""

import numpy as np
import ml_dtypes

N, E, D = 100000, 625000, 128
P = 128
NCORES = 8
NBUK = 784          # total dst buckets of 128 nodes
BPC = NBUK // NCORES  # 98 buckets per core
NPAD = NBUK * P     # 100352 padded node count
SHARD = BPC * P     # 12544 nodes per core
CHB = 7             # buckets per group (98 = 14 * 7)
NGRP = BPC // CHB   # 14
NCH = 4             # src chunks (int16 index limit)
CHUNK = NPAD // NCH  # 25088 rows per chunk

_CACHE = {}


def _prep(edge_index):
    """Host-side partitioning: group edges by (dst bucket, src chunk),
    compute degree norms, emit gather-index + one-hot-slot streams.
    Pure index bookkeeping / data movement."""
    src = edge_index[0].astype(np.int64)
    dst = edge_index[1].astype(np.int64)

    deg = np.bincount(src, minlength=N).astype(np.float64) + 1.0
    norm_pad = np.ones(NPAD, np.float32)
    norm_pad[:N] = (deg ** -0.5).astype(np.float32)
    norm2d = np.ascontiguousarray(
        norm_pad.reshape(NCORES, BPC, P).transpose(0, 2, 1)
    )  # [c, P, BPC]

    b_of = dst // P
    slot_val = (dst % P).astype(np.float32)
    core = b_of // BPC
    bl = b_of % BPC
    grp = bl // CHB
    jb = bl % CHB
    ch = src // CHUNK
    # stream order within a core: (group, chunk, bucket-in-group)
    rank = (grp * NCH + ch) * CHB + jb          # cell id in stream order
    NCELL = BPC * NCH

    key = core * NCELL + rank
    counts = np.bincount(key, minlength=NCORES * NCELL).reshape(NCORES, NCELL)
    # per-cell capacity shared across cores (uniform program)
    caps_r = np.ceil(counts.max(0) / P).astype(np.int64)      # [NCELL] rank order
    cum_r = np.zeros(NCELL + 1, np.int64)
    np.cumsum(caps_r, out=cum_r[1:])
    totE = int(cum_r[-1])

    order = np.argsort(key, kind="stable")
    k_sorted = key[order]
    starts = np.zeros(NCORES * NCELL + 1, np.int64)
    np.cumsum(counts.reshape(-1), out=starts[1:])
    pos = np.arange(len(order)) - starts[k_sorted]
    r_sorted = k_sorted % NCELL
    c_sorted = k_sorted // NCELL
    col = cum_r[r_sorted] + pos // P
    prt = pos % P

    slots = np.full((NCORES, P, totE), 999.0, np.float32)
    slots[c_sorted, prt, col] = slot_val[order]
    # flat gather positions (p-fastest within tile), chunk-relative indices
    idx_flat = np.zeros((NCORES, totE * P), np.int16)
    fpos = col * P + prt
    idx_flat[c_sorted, fpos] = (src[order] - ch[order] * CHUNK).astype(np.int16)
    # wrap to [16, totE*8] and replicate to 128 partitions
    blk = idx_flat.reshape(NCORES, totE * 8, 16).transpose(0, 2, 1)  # [c,16,S]
    idx16 = np.ascontiguousarray(np.tile(blk, (1, 8, 1)))            # [c,128,S]

    caps = caps_r.reshape(NGRP, NCH, CHB)
    cum = cum_r[:NCELL].reshape(NGRP, NCH, CHB)
    return dict(e_dst=slots, idx16=idx16, caps=caps, cum=cum, totE=totE,
                norm2d=norm2d)


def _build(caps, cum, totE, sim_single_core=False):
    import concourse.bass as bass
    import concourse.bacc as bacc
    import concourse.mybir as mybir
    import concourse.tile as tile

    F32 = mybir.dt.float32
    BF16 = mybir.dt.bfloat16
    I16 = mybir.dt.int16
    AF = mybir.ActivationFunctionType
    OP = mybir.AluOpType

    # per-group tile extents
    gstart = [int(cum[g, 0, 0]) for g in range(NGRP)]
    gend = [int(cum[g, NCH - 1, CHB - 1] + caps[g, NCH - 1, CHB - 1])
            for g in range(NGRP)]
    TGs = [gend[g] - gstart[g] for g in range(NGRP)]
    TGMAX = max(TGs)

    nc = bacc.Bacc("TRN2", target_bir_lowering=False, debug=False)
    x_sh = nc.dram_tensor("x_sh", [SHARD, D], F32, kind="ExternalInput")
    w_in = nc.dram_tensor("w_in", [D, D], F32, kind="ExternalInput")
    iota_in = nc.dram_tensor("iota_in", [P, P], BF16, kind="ExternalInput")
    iotac_in = nc.dram_tensor("iotac_in", [P, 1], F32, kind="ExternalInput")
    norm_in = nc.dram_tensor("norm_in", [P, BPC], F32, kind="ExternalInput")
    idx_in = nc.dram_tensor("idx_in", [P, totE * 8], I16, kind="ExternalInput")
    edst_in = nc.dram_tensor("edst_in", [P, totE], F32, kind="ExternalInput")
    out = nc.dram_tensor("out", [SHARD, D], F32, kind="ExternalOutput")

    def bcast(a, b):
        return bass.broadcast_tensor_aps(a, b)

    with tile.TileContext(nc) as tc:
        with (
            tc.tile_pool(name="const", bufs=1) as cst,
            tc.tile_pool(name="inp", bufs=1) as inp,
            tc.tile_pool(name="spool", bufs=2) as spool,
            tc.tile_pool(name="xgpool", bufs=2) as xgp,
            tc.tile_pool(name="gx", bufs=2) as gxp,
            tc.tile_pool(name="atp", bufs=3) as atp,
            tc.tile_pool(name="sqp", bufs=2) as sqp,
            tc.tile_pool(name="oth", bufs=2) as othp,
            tc.tile_pool(name="stage", bufs=1) as stg,
            tc.tile_pool(name="pagg", bufs=3, space="PSUM") as pa,
            tc.tile_pool(name="pw", bufs=3, space="PSUM") as pw,
            tc.tile_pool(name="dram", bufs=1, space="DRAM") as drm,
        ):
            # ---- constants ----
            iota_t = cst.tile([P, P], BF16)
            iotac_t = cst.tile([P, 1], F32)
            w_sb = cst.tile([P, P], F32)
            w_bf = cst.tile([P, P], BF16)
            ident = cst.tile([P, P], BF16)
            eps_t = cst.tile([P, 1], F32)
            nc.sync.dma_start(out=iota_t[:], in_=iota_in[:])
            nc.sync.dma_start(out=iotac_t[:], in_=iotac_in[:])
            nc.sync.dma_start(out=w_sb[:], in_=w_in[:])
            nc.vector.tensor_copy(w_bf[:], w_sb[:])
            nc.vector.tensor_scalar(
                out=ident[:], in0=iota_t[:], scalar1=iotac_t[:], scalar2=None,
                op0=OP.is_equal,
            )
            nc.gpsimd.memset(eps_t[:], 1e-30)

            # ---- input streams ----
            norm_t = inp.tile([P, BPC], F32)
            idx_t = inp.tile([P, totE * 8], I16)
            edst_t = inp.tile([P, totE], F32)
            nc.sync.dma_start(out=norm_t[:], in_=norm_in[:])
            nc.sync.dma_start(out=idx_t[:], in_=idx_in[:])
            nc.sync.dma_start(out=edst_t[:], in_=edst_in[:])

            # ---- staging ----
            ssq = stg.tile([P, BPC], F32)
            rl2 = stg.tile([P, BPC], F32)
            out_stage = stg.tile([P, BPC, P], BF16)
            gall = stg.tile([P, BPC, P], BF16)   # g rows for own nodes

            g_own = drm.tile([SHARD, D], BF16)
            g_full = drm.tile(
                [NPAD, D], BF16,
                addr_space="Local" if sim_single_core else "Shared",
            )

            # ---- phase B: g_own = norm * x (kept in SBUF), AllGather ----
            x_r = x_sh[:].rearrange("(b p) f -> p b f", p=P)
            gown_r = g_own[:].rearrange("(b p) f -> p b f", p=P)
            for grp in range(NGRP):
                sl = slice(grp * CHB, (grp + 1) * CHB)
                xch = gxp.tile([P, CHB, P], F32, tag="xch")
                nc.sync.dma_start(out=xch[:], in_=x_r[:, sl, :])
                n3 = norm_t[:, sl].rearrange("p (t a) -> p t a", a=1)
                b0, b1 = bcast(xch[:], n3)
                nc.vector.scalar_tensor_tensor(
                    out=gall[:, sl, :], in0=b0, scalar=0.0, in1=b1,
                    op0=OP.bypass, op1=OP.mult,
                )
                nc.sync.dma_start(out=gown_r[:, sl, :], in_=gall[:, sl, :])
            if sim_single_core:
                gfull_r = g_full[:].rearrange("(c s) f -> c s f", c=NCORES)
                nc.sync.dma_start(out=gfull_r[0], in_=g_own[:])
            else:
                nc.gpsimd.collective_compute(
                    "AllGather",
                    mybir.AluOpType.bypass,
                    ins=[g_own.opt()],
                    outs=[g_full.opt()],
                    replica_groups=[list(range(NCORES))],
                )

            # ---- phase C: gathers + one-hot scatter-matmul + W + ssq ----
            for grp in range(NGRP):
                g0 = gstart[grp]
                TG = TGs[grp]
                xg = xgp.tile([P, TGMAX, P], BF16, tag="xg")
                for ch in range(NCH):
                    s0 = int(cum[grp, ch, 0])
                    ntl = int(cum[grp, ch, CHB - 1] + caps[grp, ch, CHB - 1]) - s0
                    # SWDGE ring holds 1024 descriptors: cap each gather at
                    # 8 tiles (1024 rows); ring flow control pipelines them
                    off = 0
                    while off < ntl:
                        nsub = min(8, ntl - off)
                        c0 = s0 + off
                        ni = nsub * P
                        nc.gpsimd.dma_gather(
                            out_ap=xg[:, c0 - g0:c0 - g0 + nsub, :],
                            in_ap=g_full[ch * CHUNK:(ch + 1) * CHUNK, :],
                            idxs_ap=idx_t[:, c0 * 8:(c0 + nsub) * 8],
                            num_idxs=ni, num_idxs_reg=ni, elem_size=P,
                        )
                        off += nsub
                # all one-hot tiles for this group in ONE DVE op
                sBig = spool.tile([P, TGMAX, P], BF16, tag="s")
                i3 = iota_t[:].rearrange("p (a f) -> p a f", a=1)
                e3 = edst_t[:, g0:g0 + TG].rearrange("p (t a) -> p t a", a=1)
                b0, b1 = bcast(i3, e3)
                nc.vector.scalar_tensor_tensor(
                    out=sBig[:, :TG, :], in0=b0, scalar=0.0, in1=b1,
                    op0=OP.bypass, op1=OP.is_equal,
                )
                # 4 buckets share one PSUM bank
                for j0 in range(0, CHB, 4):
                    nw = min(4, CHB - j0)
                    pA = pa.tile([P, 4, P], F32, space="PSUM")
                    for q in range(nw):
                        j = j0 + q
                        bl = grp * CHB + j
                        # self-loop: dense identity matmul from resident g
                        tl = []  # (tile col within xg/sBig)
                        for ch in range(NCH):
                            c0 = int(cum[grp, ch, j])
                            for t in range(int(caps[grp, ch, j])):
                                tl.append(c0 - g0 + t)
                        nc.tensor.matmul(
                            pA[:, q, :], lhsT=gall[:, bl, :], rhs=ident[:],
                            start=True, stop=(len(tl) == 0),
                        )
                        for k, tc_ in enumerate(tl):
                            nc.tensor.matmul(
                                pA[:, q, :], lhsT=xg[:, tc_, :],
                                rhs=sBig[:, tc_, :],
                                start=False, stop=(k == len(tl) - 1),
                            )
                    at = atp.tile([P, 4, P], BF16, tag="at")
                    nc.scalar.copy(out=at[:, :nw, :], in_=pA[:, :nw, :])
                    pC = pw.tile([P, 4, P], F32, space="PSUM")
                    for q in range(nw):
                        nc.tensor.matmul(
                            pC[:, q, :], lhsT=at[:, q, :], rhs=w_bf[:],
                            start=True, stop=True,
                        )
                    bl0 = grp * CHB + j0
                    nc.scalar.copy(
                        out=out_stage[:, bl0:bl0 + nw, :], in_=pC[:, :nw, :],
                    )
                # per-group sum-of-squares: one DVE square + one DVE reduce
                sl = slice(grp * CHB, (grp + 1) * CHB)
                sq = sqp.tile([P, CHB, P], BF16, tag="sq")
                nc.vector.scalar_tensor_tensor(
                    out=sq[:], in0=out_stage[:, sl, :], scalar=0.0,
                    in1=out_stage[:, sl, :], op0=OP.bypass, op1=OP.mult,
                )
                nc.vector.tensor_reduce(
                    out=ssq[:, sl], in_=sq[:], axis=mybir.AxisListType.X,
                    op=OP.add,
                )

            # ---- final: rl2 = (ssq+eps)^-0.5; out = tanh(out_stage*rl2) ----
            nc.scalar.activation(rl2[:], ssq[:], AF.Ln, bias=eps_t[:])
            nc.scalar.activation(rl2[:], rl2[:], AF.Exp, scale=-0.5)
            out_r = out[:].rearrange("(b p) f -> p b f", p=P)
            for grp in range(NGRP):
                sl = slice(grp * CHB, (grp + 1) * CHB)
                r3 = rl2[:, sl].rearrange("p (t a) -> p t a", a=1)
                b0, b1 = bcast(out_stage[:, sl, :], r3)
                nc.vector.scalar_tensor_tensor(
                    out=out_stage[:, sl, :], in0=b0, scalar=0.0, in1=b1,
                    op0=OP.bypass, op1=OP.mult,
                )
                flat = out_stage[:, sl, :].rearrange("p t f -> p (t f)")
                oth = othp.tile([P, CHB, P], F32, tag="oth")
                nc.scalar.activation(
                    oth[:].rearrange("p t f -> p (t f)"), flat, AF.Tanh,
                )
                nc.sync.dma_start(out=out_r[:, sl, :], in_=oth[:])

    nc.compile()
    return nc


def _make_in_maps(x, W, prep):
    iota_row = np.tile(
        np.arange(P, dtype=np.float32), (P, 1)
    ).astype(ml_dtypes.bfloat16)
    iota_col = np.arange(P, dtype=np.float32).reshape(P, 1)
    x_pad = np.zeros((NPAD, D), np.float32)
    x_pad[:N] = np.asarray(x, np.float32)
    w_np = np.asarray(W, np.float32)
    in_maps = []
    for c in range(NCORES):
        in_maps.append({
            "x_sh": np.ascontiguousarray(x_pad[c * SHARD:(c + 1) * SHARD]),
            "w_in": w_np,
            "iota_in": iota_row,
            "iotac_in": iota_col,
            "norm_in": np.ascontiguousarray(prep["norm2d"][c]),
            "idx_in": np.ascontiguousarray(prep["idx16"][c]),
            "edst_in": np.ascontiguousarray(prep["e_dst"][c]),
        })
    return in_maps


def get_compiled(edge_index):
    """Build (or fetch cached) compiled program for this edge structure."""
    prep = _prep(np.asarray(edge_index))
    key = tuple(prep["caps"].reshape(-1))
    if key not in _CACHE:
        _CACHE[key] = _build(prep["caps"], prep["cum"], prep["totE"])
    return _CACHE[key], prep


def kernel(x, edge_index, W):
    from concourse.bass_utils import run_bass_kernel_spmd

    nc, prep = get_compiled(edge_index)
    in_maps = _make_in_maps(x, W, prep)
    res = run_bass_kernel_spmd(nc, in_maps, core_ids=list(range(NCORES)))
    big = np.concatenate([res.results[c]["out"] for c in range(NCORES)], axis=0)
    return np.ascontiguousarray(big[:N]).astype(np.float32)


# revision 15
# speedup vs baseline: 1.1325x; 1.1325x over previous
"""GCNConv forward on 8 Trainium2 NeuronCores (Bass/Tile).

Strategy (graph/edge-cut parallelism):
  - Nodes are split into 784 buckets of 128 (98 buckets per core); each core
    owns the scatter-sum for its node shard.
  - norm = (1+outdeg)^-0.5 is pure graph-structure metadata, precomputed on
    host alongside the edge partitioning (same category as the index streams).
  - g = norm[src] * x is computed distributed (own rows only) and shared with
    an AllGather (bf16, Shared scratchpad), so per-edge messages are plain
    row-gathers of g.
  - Edges are grouped by (dst-bucket-group, src-chunk) on host; one dma_gather
    per (group, chunk) fetches all needed g rows (int16 indices limit a chunk
    to 25088 rows), amortizing SWDGE setup over ~1-2k descriptors.
  - Each 128-edge tile scatter-adds into its bucket's PSUM via a one-hot
    matmul; all one-hot tiles of a group are built in ONE DVE op using
    zero-stride broadcast APs (iota vs dst-slot).
  - Self-loops use the dense path: g rows for own nodes stay resident in SBUF
    from phase B and enter each bucket's PSUM chain via an identity matmul.
  - 4 buckets share one PSUM bank, so one wide Activation copy moves 4
    buckets' aggregates (and later 4 matmul results) at once.
  - norm[dst] scaling cancels inside the final L2 normalization (deg >= 1
    because of self-loops), so it is skipped entirely.
  - out = tanh(L2-normalize(agg @ W)) with the L2/rsqrt done via Ln/Exp on
    the scalar engine (Rsqrt activation is banned for accuracy).
"""

import numpy as np
import ml_dtypes

N, E, D = 100000, 625000, 128
P = 128
NCORES = 8
NBUK = 784          # total dst buckets of 128 nodes
BPC = NBUK // NCORES  # 98 buckets per core
NPAD = NBUK * P     # 100352 padded node count
SHARD = BPC * P     # 12544 nodes per core
CHB = 7             # buckets per group (98 = 14 * 7)
NGRP = BPC // CHB   # 14
NCH = 4             # src chunks (int16 index limit)
CHUNK = NPAD // NCH  # 25088 rows per chunk

_CACHE = {}


def _prep(edge_index):
    """Host-side partitioning: group edges by (dst bucket, src chunk),
    compute degree norms, emit gather-index + one-hot-slot streams.
    Pure index bookkeeping / data movement."""
    src = edge_index[0].astype(np.int64)
    dst = edge_index[1].astype(np.int64)

    deg = np.bincount(src, minlength=N).astype(np.float64) + 1.0
    norm_pad = np.ones(NPAD, np.float32)
    norm_pad[:N] = (deg ** -0.5).astype(np.float32)
    norm2d = np.ascontiguousarray(
        norm_pad.reshape(NCORES, BPC, P).transpose(0, 2, 1)
    )  # [c, P, BPC]

    b_of = dst // P
    slot_val = (dst % P).astype(np.float32)
    core = b_of // BPC
    bl = b_of % BPC
    grp = bl // CHB
    jb = bl % CHB
    ch = src // CHUNK
    # stream order within a core: (group, chunk, bucket-in-group)
    rank = (grp * NCH + ch) * CHB + jb          # cell id in stream order
    NCELL = BPC * NCH

    key = core * NCELL + rank
    counts = np.bincount(key, minlength=NCORES * NCELL).reshape(NCORES, NCELL)
    # per-cell capacity shared across cores (uniform program)
    caps_r = np.ceil(counts.max(0) / P).astype(np.int64)      # [NCELL] rank order
    cum_r = np.zeros(NCELL + 1, np.int64)
    np.cumsum(caps_r, out=cum_r[1:])
    totE = int(cum_r[-1])

    order = np.argsort(key, kind="stable")
    k_sorted = key[order]
    starts = np.zeros(NCORES * NCELL + 1, np.int64)
    np.cumsum(counts.reshape(-1), out=starts[1:])
    pos = np.arange(len(order)) - starts[k_sorted]
    r_sorted = k_sorted % NCELL
    c_sorted = k_sorted // NCELL
    col = cum_r[r_sorted] + pos // P
    prt = pos % P

    slots = np.full((NCORES, P, totE), 999.0, np.float32)
    slots[c_sorted, prt, col] = slot_val[order]
    norme = np.zeros((NCORES, P, totE), np.float32)
    norme[c_sorted, prt, col] = norm_pad[src[order]]
    norme = norme.astype(ml_dtypes.bfloat16)
    # flat gather positions (p-fastest within tile), chunk-relative indices
    idx_flat = np.zeros((NCORES, totE * P), np.int16)
    fpos = col * P + prt
    idx_flat[c_sorted, fpos] = (src[order] - ch[order] * CHUNK).astype(np.int16)
    # wrap to [16, totE*8] and replicate to 128 partitions
    blk = idx_flat.reshape(NCORES, totE * 8, 16).transpose(0, 2, 1)  # [c,16,S]
    idx16 = np.ascontiguousarray(np.tile(blk, (1, 8, 1)))            # [c,128,S]

    caps = caps_r.reshape(NGRP, NCH, CHB)
    cum = cum_r[:NCELL].reshape(NGRP, NCH, CHB)
    return dict(e_dst=slots, idx16=idx16, caps=caps, cum=cum, totE=totE,
                norm2d=norm2d, norme=norme)


def _build(caps, cum, totE, sim_single_core=False):
    import concourse.bass as bass
    import concourse.bacc as bacc
    import concourse.mybir as mybir
    import concourse.tile as tile

    F32 = mybir.dt.float32
    BF16 = mybir.dt.bfloat16
    I16 = mybir.dt.int16
    AF = mybir.ActivationFunctionType
    OP = mybir.AluOpType

    # per-group tile extents
    gstart = [int(cum[g, 0, 0]) for g in range(NGRP)]
    gend = [int(cum[g, NCH - 1, CHB - 1] + caps[g, NCH - 1, CHB - 1])
            for g in range(NGRP)]
    TGs = [gend[g] - gstart[g] for g in range(NGRP)]
    TGMAX = max(TGs)

    nc = bacc.Bacc("TRN2", target_bir_lowering=False, debug=False)
    x_sh = nc.dram_tensor("x_sh", [SHARD, D], F32, kind="ExternalInput")
    x_bf = nc.dram_tensor("x_bf", [NPAD, D], BF16, kind="ExternalInput")
    w_in = nc.dram_tensor("w_in", [D, D], F32, kind="ExternalInput")
    iota_in = nc.dram_tensor("iota_in", [P, P], BF16, kind="ExternalInput")
    iotac_in = nc.dram_tensor("iotac_in", [P, 1], F32, kind="ExternalInput")
    norm_in = nc.dram_tensor("norm_in", [P, BPC], F32, kind="ExternalInput")
    idx_in = nc.dram_tensor("idx_in", [P, totE * 8], I16, kind="ExternalInput")
    edst_in = nc.dram_tensor("edst_in", [P, totE], F32, kind="ExternalInput")
    norme_in = nc.dram_tensor("norme_in", [P, totE], BF16, kind="ExternalInput")
    out = nc.dram_tensor("out", [SHARD, D], F32, kind="ExternalOutput")

    def bcast(a, b):
        return bass.broadcast_tensor_aps(a, b)

    with tile.TileContext(nc) as tc:
        with (
            tc.tile_pool(name="const", bufs=1) as cst,
            tc.tile_pool(name="inp", bufs=1) as inp,
            tc.tile_pool(name="spool", bufs=2) as spool,
            tc.tile_pool(name="xgpool", bufs=2) as xgp,
            tc.tile_pool(name="gx", bufs=2) as gxp,
            tc.tile_pool(name="atp", bufs=3) as atp,
            tc.tile_pool(name="sqp", bufs=2) as sqp,
            tc.tile_pool(name="oth", bufs=2) as othp,
            tc.tile_pool(name="stage", bufs=1) as stg,
            tc.tile_pool(name="pagg", bufs=3, space="PSUM") as pa,
            tc.tile_pool(name="pw", bufs=3, space="PSUM") as pw,
            tc.tile_pool(name="dram", bufs=1, space="DRAM") as drm,
        ):
            # ---- constants ----
            iota_t = cst.tile([P, P], BF16)
            iotac_t = cst.tile([P, 1], F32)
            w_sb = cst.tile([P, P], F32)
            w_bf = cst.tile([P, P], BF16)
            ident = cst.tile([P, P], BF16)
            eps_t = cst.tile([P, 1], F32)
            nc.sync.dma_start(out=iota_t[:], in_=iota_in[:])
            nc.sync.dma_start(out=iotac_t[:], in_=iotac_in[:])
            nc.sync.dma_start(out=w_sb[:], in_=w_in[:])
            nc.vector.tensor_copy(w_bf[:], w_sb[:])
            nc.vector.tensor_scalar(
                out=ident[:], in0=iota_t[:], scalar1=iotac_t[:], scalar2=None,
                op0=OP.is_equal,
            )
            nc.gpsimd.memset(eps_t[:], 1e-30)

            # ---- input streams ----
            norm_t = inp.tile([P, BPC], F32)
            idx_t = inp.tile([P, totE * 8], I16)
            edst_t = inp.tile([P, totE], F32)
            norme_t = inp.tile([P, totE], BF16)
            nc.sync.dma_start(out=norm_t[:], in_=norm_in[:])
            nc.sync.dma_start(out=idx_t[:], in_=idx_in[:])
            nc.sync.dma_start(out=edst_t[:], in_=edst_in[:])
            nc.sync.dma_start(out=norme_t[:], in_=norme_in[:])

            # ---- staging ----
            ssq = stg.tile([P, BPC], F32)
            rl2 = stg.tile([P, BPC], F32)
            out_stage = stg.tile([P, BPC, P], BF16)
            gall = stg.tile([P, BPC, P], BF16)   # g rows for own nodes

            # ---- phase B: self-loop rows g = norm * x stay in SBUF ----
            x_r = x_sh[:].rearrange("(b p) f -> p b f", p=P)
            for grp in range(NGRP):
                sl = slice(grp * CHB, (grp + 1) * CHB)
                xch = gxp.tile([P, CHB, P], F32, tag="xch")
                nc.sync.dma_start(out=xch[:], in_=x_r[:, sl, :])
                n3 = norm_t[:, sl].rearrange("p (t a) -> p t a", a=1)
                b0, b1 = bcast(xch[:], n3)
                nc.vector.scalar_tensor_tensor(
                    out=gall[:, sl, :], in0=b0, scalar=0.0, in1=b1,
                    op0=OP.bypass, op1=OP.mult,
                )

            # ---- phase C: gathers + one-hot scatter-matmul + W + ssq ----
            for grp in range(NGRP):
                g0 = gstart[grp]
                TG = TGs[grp]
                xg = xgp.tile([P, TGMAX, P], BF16, tag="xg")
                for ch in range(NCH):
                    s0 = int(cum[grp, ch, 0])
                    ntl = int(cum[grp, ch, CHB - 1] + caps[grp, ch, CHB - 1]) - s0
                    # SWDGE ring holds 1024 descriptors: cap each gather at
                    # 8 tiles (1024 rows); ring flow control pipelines them
                    off = 0
                    while off < ntl:
                        nsub = min(8, ntl - off)
                        c0 = s0 + off
                        ni = nsub * P
                        nc.gpsimd.dma_gather(
                            out_ap=xg[:, c0 - g0:c0 - g0 + nsub, :],
                            in_ap=x_bf[ch * CHUNK:(ch + 1) * CHUNK, :],
                            idxs_ap=idx_t[:, c0 * 8:(c0 + nsub) * 8],
                            num_idxs=ni, num_idxs_reg=ni, elem_size=P,
                        )
                        off += nsub
                # all one-hot tiles for this group in ONE DVE op
                sBig = spool.tile([P, TGMAX, P], BF16, tag="s")
                i3 = iota_t[:].rearrange("p (a f) -> p a f", a=1)
                e3 = edst_t[:, g0:g0 + TG].rearrange("p (t a) -> p t a", a=1)
                b0, b1 = bcast(i3, e3)
                nc.vector.scalar_tensor_tensor(
                    out=sBig[:, :TG, :], in0=b0, scalar=0.0, in1=b1,
                    op0=OP.bypass, op1=OP.is_equal,
                )
                # weight each edge's one-hot row by norm[src]
                ne3 = norme_t[:, g0:g0 + TG].rearrange("p (t a) -> p t a", a=1)
                c0_, c1_ = bcast(sBig[:, :TG, :], ne3)
                nc.vector.scalar_tensor_tensor(
                    out=sBig[:, :TG, :], in0=c0_, scalar=0.0, in1=c1_,
                    op0=OP.bypass, op1=OP.mult,
                )
                # 4 buckets share one PSUM bank
                for j0 in range(0, CHB, 4):
                    nw = min(4, CHB - j0)
                    pA = pa.tile([P, 4, P], F32, space="PSUM")
                    for q in range(nw):
                        j = j0 + q
                        bl = grp * CHB + j
                        # self-loop: dense identity matmul from resident g
                        tl = []  # (tile col within xg/sBig)
                        for ch in range(NCH):
                            c0 = int(cum[grp, ch, j])
                            for t in range(int(caps[grp, ch, j])):
                                tl.append(c0 - g0 + t)
                        nc.tensor.matmul(
                            pA[:, q, :], lhsT=gall[:, bl, :], rhs=ident[:],
                            start=True, stop=(len(tl) == 0),
                        )
                        for k, tc_ in enumerate(tl):
                            nc.tensor.matmul(
                                pA[:, q, :], lhsT=xg[:, tc_, :],
                                rhs=sBig[:, tc_, :],
                                start=False, stop=(k == len(tl) - 1),
                            )
                    at = atp.tile([P, 4, P], BF16, tag="at")
                    nc.scalar.copy(out=at[:, :nw, :], in_=pA[:, :nw, :])
                    pC = pw.tile([P, 4, P], F32, space="PSUM")
                    for q in range(nw):
                        nc.tensor.matmul(
                            pC[:, q, :], lhsT=at[:, q, :], rhs=w_bf[:],
                            start=True, stop=True,
                        )
                    bl0 = grp * CHB + j0
                    nc.scalar.copy(
                        out=out_stage[:, bl0:bl0 + nw, :], in_=pC[:, :nw, :],
                    )
                # per-group sum-of-squares: one DVE square + one DVE reduce
                sl = slice(grp * CHB, (grp + 1) * CHB)
                sq = sqp.tile([P, CHB, P], BF16, tag="sq")
                nc.vector.scalar_tensor_tensor(
                    out=sq[:], in0=out_stage[:, sl, :], scalar=0.0,
                    in1=out_stage[:, sl, :], op0=OP.bypass, op1=OP.mult,
                )
                nc.vector.tensor_reduce(
                    out=ssq[:, sl], in_=sq[:], axis=mybir.AxisListType.X,
                    op=OP.add,
                )

            # ---- final: rl2 = (ssq+eps)^-0.5; out = tanh(out_stage*rl2) ----
            nc.scalar.activation(rl2[:], ssq[:], AF.Ln, bias=eps_t[:])
            nc.scalar.activation(rl2[:], rl2[:], AF.Exp, scale=-0.5)
            out_r = out[:].rearrange("(b p) f -> p b f", p=P)
            for grp in range(NGRP):
                sl = slice(grp * CHB, (grp + 1) * CHB)
                r3 = rl2[:, sl].rearrange("p (t a) -> p t a", a=1)
                b0, b1 = bcast(out_stage[:, sl, :], r3)
                nc.vector.scalar_tensor_tensor(
                    out=out_stage[:, sl, :], in0=b0, scalar=0.0, in1=b1,
                    op0=OP.bypass, op1=OP.mult,
                )
                flat = out_stage[:, sl, :].rearrange("p t f -> p (t f)")
                oth = othp.tile([P, CHB, P], F32, tag="oth")
                nc.scalar.activation(
                    oth[:].rearrange("p t f -> p (t f)"), flat, AF.Tanh,
                )
                nc.sync.dma_start(out=out_r[:, sl, :], in_=oth[:])

    nc.compile()
    return nc


def _make_in_maps(x, W, prep):
    iota_row = np.tile(
        np.arange(P, dtype=np.float32), (P, 1)
    ).astype(ml_dtypes.bfloat16)
    iota_col = np.arange(P, dtype=np.float32).reshape(P, 1)
    x_pad = np.zeros((NPAD, D), np.float32)
    x_pad[:N] = np.asarray(x, np.float32)
    x_bf = x_pad.astype(ml_dtypes.bfloat16)
    w_np = np.asarray(W, np.float32)
    in_maps = []
    for c in range(NCORES):
        in_maps.append({
            "x_sh": np.ascontiguousarray(x_pad[c * SHARD:(c + 1) * SHARD]),
            "x_bf": x_bf,
            "w_in": w_np,
            "iota_in": iota_row,
            "iotac_in": iota_col,
            "norm_in": np.ascontiguousarray(prep["norm2d"][c]),
            "idx_in": np.ascontiguousarray(prep["idx16"][c]),
            "edst_in": np.ascontiguousarray(prep["e_dst"][c]),
            "norme_in": np.ascontiguousarray(prep["norme"][c]),
        })
    return in_maps


def get_compiled(edge_index):
    """Build (or fetch cached) compiled program for this edge structure."""
    prep = _prep(np.asarray(edge_index))
    key = tuple(prep["caps"].reshape(-1))
    if key not in _CACHE:
        _CACHE[key] = _build(prep["caps"], prep["cum"], prep["totE"])
    return _CACHE[key], prep


def kernel(x, edge_index, W):
    from concourse.bass_utils import run_bass_kernel_spmd

    nc, prep = get_compiled(edge_index)
    in_maps = _make_in_maps(x, W, prep)
    res = run_bass_kernel_spmd(nc, in_maps, core_ids=list(range(NCORES)))
    big = np.concatenate([res.results[c]["out"] for c in range(NCORES)], axis=0)
    return np.ascontiguousarray(big[:N]).astype(np.float32)
